# revision 20
# baseline (speedup 1.0000x reference)
"""TRN2 Bass kernel for nn_CortexNetwork (dense_cnn).

Computation (see reference):
  afferent[b,i,j] = sum_{h,w} input[b, i+h, j+w] * AW[i,j,h,w]   (locally connected)
  total = afferent + 0.9 * prev @ W_e.T - 0.9 * prev @ W_i.T
  out = relu(total)                                               # [B=8, N=9216]

Sharding: output units (grid rows i) split across 8 cores -> 12 rows / 1152
units per core. Each core streams its [N, 1152] transposed chunk of W_e/W_i
(host-transposed so the contraction dim lands on SBUF partitions).

Lateral matmuls use an fp32-accurate double-fp16 decomposition: every f32
weight w is stored as (hi=fp16(w), lo=fp16((w-hi)*2^11)) — same 4 bytes of
HBM traffic as f32, but fp16 matmuls stream 1 column/cycle vs fp32's 4.
With x split the same way, total = x@W ~= xhi@Whi + 2^-11*(xlo_s@Whi +
xhi@Wlo_s), dropping the ~2^-22 lo*lo term.  The Whi pass computes both of
its terms in one M=16 matmul (stationary [xhi; xlo_s]) into PSUM rows 0..15;
the Wlo pass adds into rows 8..15.  Rows 8..15 are scaled by 2^-11 (ACT),
shifted to partitions 0..7 by a small SBUF->SBUF DMA, and added to rows 0..7.
Measured accuracy of this scheme vs float64 matches native fp32 (~4e-7).

The afferent term is computed on DVE (elementwise mult + reduce) over
host-pre-shifted input copies (partitions = j, one shifted copy per w
offset, since compute engines cannot move data across partitions), then
injected into the lateral PSUM rows 0..7 with accumulating PE transposes.
"""

import numpy as np

from concourse import bacc
import concourse.mybir as mybir
from concourse.tile import TileContext
from concourse.bass_utils import run_bass_kernel_spmd
from concourse.masks import make_identity

GX = GY = 96
RF = 24
B = 8
N = GX * GY            # 9216
NCORES = 8
IPC = GX // NCORES     # 12 grid rows per core
UC = IPC * GY          # 1152 output units per core
ROWS = IPC - 1 + RF    # 35 input rows a core needs
KBLK = N // 128        # 72 contraction blocks of 128
G = 4                  # k-blocks per DMA slab (steady state)
UB = 384               # unit-block width (one PSUM bank each)
NUB = UC // UB         # 3 unit blocks
GAMMA = 0.9
LO_SCALE = 2.0 ** 11   # lo parts are stored pre-multiplied by this
F32 = mybir.dt.float32
F16 = mybir.dt.float16

_PROGRAM = None


def _build_program():
    nc = bacc.Bacc(trn_type="TRN2")
    # W parts packed per k-row: [N, part(hi,lo), UC] fp16
    wte = nc.dram_tensor("wte", [N, 2 * UC], F16, kind="ExternalInput")
    wti = nc.dram_tensor("wti", [N, 2 * UC], F16, kind="ExternalInput")
    # lhs[p, m, kb, 0:8]=xlo_m (scaled), [.., 8:32]=0, [.., 32:40]=xhi_m
    lhs = nc.dram_tensor("lhs", [128, 2 * KBLK * 40], F16, kind="ExternalInput")
    shin = nc.dram_tensor("shin", [GY, RF * B * ROWS], F32, kind="ExternalInput")
    wa = nc.dram_tensor("wa", [GY, IPC * RF * RF], F32, kind="ExternalInput")
    out = nc.dram_tensor("out", [B, UC], F32, kind="ExternalOutput")

    with TileContext(nc) as tc:
        with tc.tile_pool(name="const", bufs=1) as cpool, \
             tc.tile_pool(name="wstream", bufs=5) as wpool, \
             tc.tile_pool(name="work", bufs=1) as wkpool, \
             tc.tile_pool(name="psum", bufs=1, space="PSUM") as psum_pool:

            # afferent-side inputs go over the ACT HWDGE ring so the slab
            # stream on SP starts immediately
            lhs_sb = cpool.tile([128, 2 * KBLK * 40], F16)
            nc.scalar.dma_start(out=lhs_sb, in_=lhs.ap())
            shin_sb = cpool.tile([GY, RF * B * ROWS], F32)
            nc.scalar.dma_start(out=shin_sb, in_=shin.ap())
            wa_sb = cpool.tile([GY, IPC * RF * RF], F32)
            nc.scalar.dma_start(out=wa_sb, in_=wa.ap())
            identity = cpool.tile([128, 128], F32)
            make_identity(nc, identity)
            aff_sb = cpool.tile([GY, B * IPC], F32)

            lhs_v = lhs_sb.rearrange("p (m kb x) -> p m kb x", m=2, kb=KBLK)
            assert lhs_v.shape[3] == 40
            # shin[j, w, b, r] = input[b, base+r, j+w]
            shin_v = shin_sb.rearrange("j (w b r) -> j w b r", w=RF, b=B)
            # wa[j, i, w, h] = AW[base_i+i, j, h, w]
            wa_v = wa_sb.rearrange("j (i w h) -> j i w h", i=IPC, w=RF)

            # --- afferent: elementwise mult per (b, i), reduce over (w, h)
            QTR = IPC // 4
            for b in range(B):
                for quarter in range(4):
                    mulbuf = wkpool.tile([GY, QTR, RF, RF], F32, name="mulbuf",
                                         tag="mulbuf")
                    for ii in range(QTR):
                        i = quarter * QTR + ii
                        nc.vector.tensor_tensor(
                            out=mulbuf[:, ii],
                            in0=shin_v[:, :, b, i:i + RF],
                            in1=wa_v[:, i],
                            op=mybir.AluOpType.mult,
                        )
                    nc.vector.tensor_reduce(
                        out=aff_sb[:, b * IPC + quarter * QTR:
                                   b * IPC + (quarter + 1) * QTR],
                        in_=mulbuf,
                        axis=mybir.AxisListType.XY,
                        op=mybir.AluOpType.add,
                    )

            # --- lateral: rows 0:8 accumulate the scaled set S = xlo_s@Whi
            #     + xhi@Wlo_s (+ afferent*2^11 via transposes); rows 8:16
            #     accumulate A = xhi@Whi.  PSUM-writing matmuls must start
            #     at partition 0/32/64, hence S in rows 0:8 and A in rows
            #     32:40 (stationary cols 8:32 are zeros).
            psums = [psum_pool.tile([40, UB], F32, name=f"pslat{u}")
                     for u in range(NUB)]

            # first chunks are small so PE starts within ~2us; padded shape
            # keeps every chunk in the same G-sized pool slot
            chunks = [(0, 1), (1, 1), (2, 2)] + [(k, G) for k in range(4, KBLK, G)]
            for m, wt in enumerate((wte, wti)):
                for (k0, ng) in (chunks if m == 0 else
                                 [(k, G) for k in range(0, KBLK, G)]):
                    slab = wpool.tile([128, ng, 2, UC], F16, name="slab",
                                      tag="slab",
                                      padded_shape=[128, G, 2, UC])
                    src = wt.ap()[k0 * 128:(k0 + ng) * 128, :]
                    nc.sync.dma_start(
                        out=slab,
                        in_=src.rearrange("(kb p) x -> p kb x", p=128)
                               .rearrange("p kb (part u) -> p kb part u",
                                          part=2))
                    for kk in range(ng):
                        kb = k0 + kk
                        first = (m == 0 and kb == 0)
                        last = (m == 1 and kb == KBLK - 1)
                        for u in range(NUB):
                            us = slice(u * UB, (u + 1) * UB)
                            # `stop` closes the sim's group bookkeeping for
                            # the whole 0:40 row range; the later row-0:8
                            # writers skip the group check (stop is a no-op
                            # on hardware).
                            nc.tensor.matmul(
                                psums[u],
                                lhs_v[:, m, kb, :],
                                slab[:, kk, 0, us],
                                start=first,
                                stop=last,
                            )
                            nc.tensor.matmul(
                                psums[u][0:8, :],
                                lhs_v[:, m, kb, 32:40],
                                slab[:, kk, 1, us],
                                start=False,
                                stop=False,
                                skip_group_check=True,
                            )

            # --- inject afferent into PSUM rows 0:8: accumulate aff.T
            aff_bv = aff_sb.rearrange("j (b i) -> j b i", b=B)
            per_u = IPC // NUB
            for i in range(IPC):
                u, off = divmod(i * GY, UB)
                nc.tensor.matmul(
                    psums[u][0:8, off:off + GY],
                    aff_bv[:, :, i],
                    identity[:GY, :GY],
                    is_transpose=True,
                    start=False,
                    stop=False,
                    skip_group_check=True,
                )

            # --- epilogue: total = 2^-11 * S + A.  A (rows 8:16) is copied
            # to SBUF and shifted to partitions 0:8 by a small SBUF->SBUF
            # DMA; then one fused multiply-add against S, relu, store.
            out_sb = cpool.tile([B, UC], F32)
            for u in range(NUB):
                us = slice(u * UB, (u + 1) * UB)
                tmp_hi = cpool.tile([40, UB], F32, name=f"tmp_hi{u}")
                nc.scalar.copy(tmp_hi[32:40, :], psums[u][32:40, :])
                tmp_a = cpool.tile([B, UB], F32, name=f"tmp_a{u}")
                nc.sync.dma_start(out=tmp_a, in_=tmp_hi[32:40, :])
                nc.vector.scalar_tensor_tensor(
                    out=out_sb[:, us],
                    in0=psums[u][0:8, :],
                    scalar=float(1.0 / LO_SCALE),
                    in1=tmp_a,
                    op0=mybir.AluOpType.mult,
                    op1=mybir.AluOpType.add,
                )
                nc.scalar.activation(out=out_sb[:, us], in_=out_sb[:, us],
                                     func=mybir.ActivationFunctionType.Relu)
            nc.sync.dma_start(out=out.ap(), in_=out_sb)

    nc.finalize()
    return nc


def _split_f16(a):
    """f32 array -> (hi, lo) float16 with lo pre-scaled by 2^11.
    hi subnormals are flushed on the host so device FTZ behavior is moot."""
    hi = a.astype(np.float16)
    hi = np.where(np.abs(hi.astype(np.float32)) < 2.0 ** -14,
                  np.float16(0), hi)
    lo = ((a - hi.astype(np.float32)) * np.float32(LO_SCALE)).astype(np.float16)
    lo = np.where(np.abs(lo.astype(np.float32)) < 2.0 ** -14,
                  np.float16(0), lo)
    return hi, lo


def _prep_in_maps(input, prev_activity, afferent_weights, W_e, W_i):
    input = np.ascontiguousarray(np.asarray(input, dtype=np.float32))
    prev = np.asarray(prev_activity, dtype=np.float32)
    aw = np.asarray(afferent_weights, dtype=np.float32)
    W_e = np.asarray(W_e, dtype=np.float32)
    W_i = np.asarray(W_i, dtype=np.float32)

    # lhs[p, m, kb, 0:8]=xlo_scaled, [.., 8:32]=0, [.., 32:40]=xhi
    lhs = np.zeros((128, 2, KBLK, 40), np.float16)
    for m, gam in enumerate((GAMMA, -GAMMA)):
        xhi, xlo = _split_f16((gam * prev).T)            # [N, B]
        lhs[:, m, :, 0:8] = xlo.reshape(KBLK, 128, B).transpose(1, 0, 2)
        lhs[:, m, :, 32:40] = xhi.reshape(KBLK, 128, B).transpose(1, 0, 2)
    lhs = np.ascontiguousarray(lhs.reshape(128, 2 * KBLK * 40))

    in_maps = []
    for c in range(NCORES):
        rows = input[:, IPC * c:IPC * c + ROWS, :]       # [B, ROWS, 119]
        shin = np.empty((GY, RF, B, ROWS), np.float32)
        for w in range(RF):
            shin[:, w] = rows[:, :, w:w + GY].transpose(2, 0, 1)
        shin = shin.reshape(GY, RF * B * ROWS)
        # pre-scaled by 2^11: the afferent term lands in the scaled PSUM
        # row set and is divided back down in the epilogue
        wa_c = np.ascontiguousarray(
            aw[IPC * c:IPC * (c + 1)].transpose(1, 0, 3, 2)
            .reshape(GY, IPC * RF * RF)) * np.float32(LO_SCALE)
        wmaps = {}
        for name, W in (("wte", W_e), ("wti", W_i)):
            hi, lo = _split_f16(W[c * UC:(c + 1) * UC, :].T)   # [N, UC]
            wmaps[name] = np.ascontiguousarray(
                np.stack([hi, lo], axis=1).reshape(N, 2 * UC))
        in_maps.append(
            {**wmaps, "lhs": lhs, "shin": shin, "wa": wa_c})
    return in_maps


def get_program():
    global _PROGRAM
    if _PROGRAM is None:
        _PROGRAM = _build_program()
    return _PROGRAM


def kernel(**inputs) -> np.ndarray:
    nc = get_program()
    in_maps = _prep_in_maps(
        inputs["input"], inputs["prev_activity"], inputs["afferent_weights"],
        inputs["W_e"], inputs["W_i"])
    res = run_bass_kernel_spmd(nc, in_maps, list(range(NCORES)))
    return np.concatenate(
        [res.results[c]["out"] for c in range(NCORES)], axis=1)
